# revision 11
# baseline (speedup 1.0000x reference)
"""Trainium2 Bass kernel for nn_Attention (pooling attention).

Math (per batch b):
    u[b]     = W_score @ h_t[b]            (score = (hidden @ W_score) . h_t
                                            collapses to hidden . (W_score @ h_t))
    score[t] = hidden[b,t,:] . u[b]        (DVE f16 mul/adds + GPSIMD add + DVE reduce)
    p[t]     = exp(score[t] - 50)          (ScalarE, bf16 out, fused accum -> q)
    s        = sum_t p[t]                  (tiny PE ones-matmul over q - no gpsimd
                                            custom ops, they thrash pool config)
    ctx      = sum_t p[t] * hidden[b,t,:]  (PE bf16xf16 matmuls, UNNORMALIZED -
                                            bf16 exp can't overflow)
    ctxT     = scatter of ctx * (1/s)      (normalization folded into the tiny
                                            transpose-scatter matmul's rhs)
    out[b]   = tanh([ctx, h_t[b]] @ W_att)

hidden_states is staged to HBM as fp16 host-side (the numerics the device sees
are identical to a cast-during-DMA of the fp32 input, which is what the
previous version did - but HBM traffic halves: 16.8MB/core -> ~47us flood
roofline).  h_t rides along as a small fp32 side input so the u-chain and the
final concat keep full precision.  The flood rides the HWDGE sync ring; setup
loads ride the scalar ring so nothing delays the flood.  Softmax
normalization is off the critical path (folded into the scatter), so the
score->ctx pipeline streams without any cross-batch serialization.

Sharding: data-parallel over batch, 16 batches per core on 8 cores; weights
replicated.
"""

import sys

import numpy as np

_TRN_REPO = "/opt/trn_rl_repo"
if _TRN_REPO not in sys.path:
    sys.path.insert(0, _TRN_REPO)

import concourse.bass as bass
import concourse.bacc as bacc
import concourse.tile as tile
from concourse import mybir
from concourse import bass_isa
from concourse.bass_utils import run_bass_kernel_spmd

N_CORES = 8
B, T, H = 128, 2048, 256
NB = B // N_CORES  # batches per core
P = 128  # SBUF partitions
TT = T // P  # t-tiles per batch (16)
HT = TT // 2  # t-tiles per half (8)
OUT_D = 128
EXP_SHIFT = -50.0  # keeps exp() in fp32/bf16 range; cancels in the softmax ratio

F32 = mybir.dt.float32
F16 = mybir.dt.float16
BF16 = mybir.dt.bfloat16


def _build_kernel(nc: bass.Bass, tc: "tile.TileContext", hidden, ht32, wst, watt, ident, out):
    add = mybir.AluOpType.add

    from contextlib import ExitStack

    with ExitStack() as ctx:
        const = ctx.enter_context(tc.tile_pool(name="const", bufs=1))
        ybufs = ctx.enter_context(tc.tile_pool(name="ybufs", bufs=8))
        sc = ctx.enter_context(tc.tile_pool(name="sc", bufs=2))
        psum_t = ctx.enter_context(tc.tile_pool(name="psum_t", bufs=3, space="PSUM"))
        psum_u = ctx.enter_context(tc.tile_pool(name="psum_u", bufs=2, space="PSUM"))
        psum_p = ctx.enter_context(tc.tile_pool(name="psum_p", bufs=1, space="PSUM"))

        # ---- setup loads: scalar (ACT) HWDGE ring, so they bypass the flood's
        # sync-ring FIFO and complete while the first y-loads stream.
        ident_sb = const.tile([16, 16], F32, tag="ident")
        nc.scalar.dma_start(out=ident_sb, in_=ident[:, :])
        ht_sb = const.tile([NB, H], F32, tag="ht")
        nc.scalar.dma_start(out=ht_sb, in_=ht32[:, :])
        wst_sb = const.tile([P, 2, H], F32, tag="wst")  # W_score^T as [k, kk, h]
        nc.scalar.dma_start(out=wst_sb, in_=wst.rearrange("(kk p) h -> p kk h", p=P))
        watt_sb = const.tile([P, 4, OUT_D], F32, tag="watt")  # W_att as [d, dd, j]
        nc.scalar.dma_start(out=watt_sb, in_=watt.rearrange("(dd p) j -> p dd j", p=P))

        ones_row16 = const.tile([1, P], F16, tag="ones_row16")
        nc.vector.memset(ones_row16, 1.0)
        ones_col2 = const.tile([P, 2], F32, tag="ones_col2")
        nc.vector.memset(ones_col2, 1.0)
        shift_col = const.tile([P, 1], F32, tag="shift_col")
        nc.vector.memset(shift_col, EXP_SHIFT)

        # ---- y-load flood: sync HWDGE ring, 32 half-batch f16 tiles --------
        # t = p*TT + i block mapping gives contiguous 4KB runs per partition
        # (softmax/context are t-permutation-invariant, so relabeling is free).
        ylist = {}

        def emit_load(k):
            b, v = divmod(k, 2)
            y = ybufs.tile([P, HT, H], F16, tag="y16", name=f"y16_{k}")
            src = hidden[b].rearrange("(p i) h -> p i h", i=TT)[
                :, v * HT : (v + 1) * HT, :
            ]
            nc.sync.dma_start(out=y, in_=src)
            ylist[k] = y

        PREF = 6  # half-tiles of DMA-ahead
        for k in range(PREF):
            emit_load(k)

        # ---- h_t^T and u = h_t @ W_score^T (full fp32) ---------------------
        htT_sb = const.tile([P, 2, NB], F32, tag="htT")  # h_t^T halves [k, half, b]
        for half in range(2):
            ps_tr = psum_t.tile([P, NB], F32, tag="ptmp", name=f"ps_tr{half}")
            nc.tensor.matmul(
                ps_tr,
                lhsT=ht_sb[:, half * P : (half + 1) * P],
                rhs=ident_sb,
                start=True,
                stop=True,
            )
            nc.scalar.copy(out=htT_sb[:, half, :], in_=ps_tr)

        ps_u = psum_t.tile([NB, H], F32, tag="ptmp")
        for half in range(2):
            nc.tensor.matmul(
                ps_u,
                lhsT=htT_sb[:, half, :],
                rhs=wst_sb[:, half, :],
                start=(half == 0),
                stop=(half == 1),
            )
        u16_sb = const.tile([NB, H], F16, tag="u16")
        nc.scalar.copy(out=u16_sb, in_=ps_u)
        # flatten u onto partition 0 so the per-batch broadcast matmul's rhs
        # has a legal base partition (small SBUF->SBUF DMA, scalar ring)
        u16_flat = const.tile([1, NB, H], F16, tag="u16_flat")
        nc.scalar.dma_start(out=u16_flat, in_=u16_sb)

        # ---- persistent PSUM accumulators ----------------------------------
        ctxT_ps = [
            psum_p.tile([P, NB], F32, tag=f"ctxT{j}", name=f"ctxT{j}")
            for j in range(2)
        ]
        # final output accumulator: do the h_t @ W_att half at setup time
        out_ps = psum_p.tile([NB, OUT_D], F32, tag="out_ps", name="out_ps")
        for dd in range(2, 4):
            nc.tensor.matmul(
                out_ps,
                lhsT=htT_sb[:, dd - 2, :],
                rhs=watt_sb[:, dd, :],
                start=(dd == 2),
                stop=False,
            )

        # ---- per-batch pipeline, tail stages skewed by one batch -----------
        state = {}

        def emit_batch_head(b):
            p_t = sc.tile([P, TT], BF16, tag="p", name=f"p{b}")
            q2 = sc.tile([P, 2], F32, tag="q2", name=f"q2{b}")
            ctx_ps = psum_t.tile([1, H], F32, tag="ptmp", name=f"ctx{b}")

            # broadcast u[b] to all partitions (PE ones-matmul), then
            # materialize it replicated HT times as a PLAIN f16 tile so the
            # score mul's in1 has no 0-stride dim (keeps DVE 2x mode).
            ubc_ps = psum_u.tile([P, H], F32, tag="ubc", name=f"ubc{b}")
            nc.tensor.matmul(
                ubc_ps,
                lhsT=ones_row16,
                rhs=u16_flat[:, b, :],
                start=True,
                stop=True,
            )
            ubc8 = sc.tile([P, HT, H], F16, tag="ubc8", name=f"ubc8_{b}")
            nc.scalar.copy(
                out=ubc8, in_=ubc_ps.unsqueeze(1).broadcast_to([P, HT, H])
            )

            for v in range(2):
                k = b * 2 + v
                if k + PREF < 2 * NB:
                    emit_load(k + PREF)
                y16 = ylist.pop(k)

                # score chain: DVE mul (f16 2x), GPSIMD pair-add, DVE pair-add
                # + reduce
                z = sc.tile([P, HT, H], F16, tag="z")
                nc.vector.tensor_mul(z, y16, ubc8)
                z1 = sc.tile([P, HT, 128], F16, tag="z1")
                nc.gpsimd.tensor_add(z1, z[:, :, 0:128], z[:, :, 128:256])
                z2 = sc.tile([P, HT, 64], F16, tag="z2")
                nc.vector.tensor_add(z2, z1[:, :, 0:64], z1[:, :, 64:128])
                score = sc.tile([P, HT], F32, tag="score")
                nc.vector.tensor_reduce(
                    out=score, in_=z2, axis=mybir.AxisListType.X, op=add
                )

                # p = exp(score - 50) in bf16 (can't overflow), q_v = sum of p
                nc.scalar.activation(
                    out=p_t[:, v * HT : (v + 1) * HT],
                    in_=score,
                    func=mybir.ActivationFunctionType.Exp,
                    bias=shift_col,
                    scale=1.0,
                    accum_out=q2[:, v : v + 1],
                )

                # ctx += sum_i p[:, i] * y[:, i, :]  (unnormalized)
                for i in range(HT):
                    ii = v * HT + i
                    nc.tensor.matmul(
                        ctx_ps,
                        lhsT=p_t[:, ii : ii + 1],
                        rhs=y16[:, i, :],
                        start=(ii == 0),
                        stop=(ii == TT - 1),
                    )
            state[b] = (q2, ctx_ps)

        def emit_batch_tail(b):
            # s = sum over partitions of q via a tiny PE ones-matmul (gpsimd
            # partition_all_reduce thrashes the pool config - avoid).
            # Normalization is folded into the scatter matmul's rhs.  Emitted
            # one batch late so these ops never sit ahead of the next batch's
            # streaming stages in any engine's instruction stream.
            q2, ctx_ps = state.pop(b)
            q = sc.tile([P, 1], F32, tag="q")
            nc.vector.tensor_add(q, q2[:, 0:1], q2[:, 1:2])
            s_ps = psum_t.tile([2, 1], F32, tag="ptmp", name=f"s{b}")
            nc.tensor.matmul(s_ps, lhsT=ones_col2, rhs=q, start=True, stop=True)
            s_sb = sc.tile([2, 1], F32, tag="s_sb")
            nc.scalar.copy(out=s_sb, in_=s_ps)
            rs2 = sc.tile([2, 1], F32, tag="rs")
            nc.vector.reciprocal(out=rs2, in_=s_sb)

            ctx_row = sc.tile([1, H], F32, tag="ctx_row_sb")
            nc.scalar.copy(out=ctx_row, in_=ctx_ps)
            for j in range(2):
                nc.tensor.matmul(
                    ctxT_ps[j][:, b : b + 1],
                    lhsT=ctx_row[:, j * P : (j + 1) * P],
                    rhs=rs2[0:1, :],
                    start=True,
                    stop=True,
                )

        for b in range(NB):
            emit_batch_head(b)
            if b >= 1:
                emit_batch_tail(b - 1)
        emit_batch_tail(NB - 1)

        # ---- finalize: ctx^T @ W_att (accumulate onto h_t part), tanh ------
        preT = sc.tile([P, 2, NB], F32, tag="preT")
        for j in range(2):
            nc.scalar.copy(out=preT[:, j, :], in_=ctxT_ps[j])
        for dd in range(2):
            nc.tensor.matmul(
                out_ps,
                lhsT=preT[:, dd, :],
                rhs=watt_sb[:, dd, :],
                start=False,
                stop=(dd == 1),
            )
        out_sb = sc.tile([NB, OUT_D], F32, tag="out_sb")
        nc.scalar.activation(
            out=out_sb, in_=out_ps, func=mybir.ActivationFunctionType.Tanh
        )
        nc.sync.dma_start(out=out[:, :], in_=out_sb)


_NC_CACHE = {}


def _get_nc():
    if "nc" not in _NC_CACHE:
        nc = bacc.Bacc("TRN2", target_bir_lowering=False, debug=False)
        hidden = nc.declare_dram_parameter("hidden", [NB, T, H], F16, isOutput=False)
        ht32 = nc.declare_dram_parameter("ht32", [NB, H], F32, isOutput=False)
        wst = nc.declare_dram_parameter("w_score_t", [H, H], F32, isOutput=False)
        watt = nc.declare_dram_parameter("w_att", [2 * H, OUT_D], F32, isOutput=False)
        ident = nc.declare_dram_parameter("ident16", [16, 16], F32, isOutput=False)
        out = nc.declare_dram_parameter("out", [NB, OUT_D], F32, isOutput=True)
        with tile.TileContext(nc) as tc:
            _build_kernel(nc, tc, hidden, ht32, wst, watt, ident, out)
        nc.compile()
        _NC_CACHE["nc"] = nc
    return _NC_CACHE["nc"]


def _run(hidden_states, W_score, W_att, trace=False, trace_kwargs=None):
    hidden_states = np.asarray(hidden_states, dtype=np.float32)
    W_score = np.asarray(W_score, dtype=np.float32)
    W_att = np.ascontiguousarray(np.asarray(W_att, dtype=np.float32))
    hidden16 = np.ascontiguousarray(hidden_states.astype(np.float16))
    ht32 = np.ascontiguousarray(hidden_states[:, T - 1, :])
    wst = np.ascontiguousarray(W_score.T)
    ident = np.eye(16, dtype=np.float32)

    nc = _get_nc()
    in_maps = []
    for c in range(N_CORES):
        in_maps.append(
            {
                "hidden": hidden16[c * NB : (c + 1) * NB],
                "ht32": ht32[c * NB : (c + 1) * NB],
                "w_score_t": wst,
                "w_att": W_att,
                "ident16": ident,
            }
        )
    kwargs = {}
    if trace:
        kwargs["trace"] = True
        if trace_kwargs:
            kwargs.update(trace_kwargs)
    res = run_bass_kernel_spmd(nc, in_maps, list(range(N_CORES)), **kwargs)
    out = np.concatenate([res.results[c]["out"] for c in range(N_CORES)], axis=0)
    return out, res


def kernel(hidden_states, W_score, W_att):
    out, _ = _run(hidden_states, W_score, W_att, trace=False)
    return out


# revision 14
# speedup vs baseline: 1.0955x; 1.0955x over previous
"""Trainium2 Bass kernel for nn_Attention (pooling attention).

Math (per batch b):
    u[b]     = W_score @ h_t[b]            (score = (hidden @ W_score) . h_t
                                            collapses to hidden . (W_score @ h_t))
    score[t] = hidden[b,t,:] . u[b]        (DVE f16 mul/adds + GPSIMD add + DVE reduce)
    p[t]     = exp(score[t] - 50)          (ScalarE, bf16 out, fused accum -> q)
    s        = sum_t p[t]                  (tiny PE ones-matmul over q - no gpsimd
                                            custom ops, they thrash pool config)
    ctx      = sum_t p[t] * hidden[b,t,:]  (PE bf16xf16 matmuls, UNNORMALIZED -
                                            bf16 exp can't overflow)
    ctxT     = scatter of ctx * (1/s)      (normalization folded into the tiny
                                            transpose-scatter matmul's rhs)
    out[b]   = tanh([ctx, h_t[b]] @ W_att)

hidden_states is staged to HBM as fp16 host-side (the numerics the device sees
are identical to a cast-during-DMA of the fp32 input, which is what the
previous version did - but HBM traffic halves: 16.8MB/core -> ~47us flood
roofline).  h_t rides along as a small fp32 side input so the u-chain and the
final concat keep full precision.  The flood rides the HWDGE sync ring; setup
loads ride the scalar ring so nothing delays the flood.  Softmax
normalization is off the critical path (folded into the scatter), so the
score->ctx pipeline streams without any cross-batch serialization.

Sharding: data-parallel over batch, 16 batches per core on 8 cores; weights
replicated.
"""

import sys

import numpy as np

_TRN_REPO = "/opt/trn_rl_repo"
if _TRN_REPO not in sys.path:
    sys.path.insert(0, _TRN_REPO)

import concourse.bass as bass
import concourse.bacc as bacc
import concourse.tile as tile
from concourse import mybir
from concourse import bass_isa
from concourse.bass_utils import run_bass_kernel_spmd

N_CORES = 8
B, T, H = 128, 2048, 256
NB = B // N_CORES  # batches per core
P = 128  # SBUF partitions
TT = T // P  # t-tiles per batch (16)
HT = TT // 2  # t-tiles per half (8)
OUT_D = 128
EXP_SHIFT = -50.0  # keeps exp() in fp32/bf16 range; cancels in the softmax ratio

F32 = mybir.dt.float32
F16 = mybir.dt.float16
BF16 = mybir.dt.bfloat16


def _build_kernel(nc: bass.Bass, tc: "tile.TileContext", hidden, ht32, wst, watt, ident, out):
    add = mybir.AluOpType.add

    from contextlib import ExitStack

    with ExitStack() as ctx:
        const = ctx.enter_context(tc.tile_pool(name="const", bufs=1))
        ybufs = ctx.enter_context(tc.tile_pool(name="ybufs", bufs=8))
        sc = ctx.enter_context(tc.tile_pool(name="sc", bufs=2))
        psum_t = ctx.enter_context(tc.tile_pool(name="psum_t", bufs=3, space="PSUM"))
        psum_u = ctx.enter_context(tc.tile_pool(name="psum_u", bufs=2, space="PSUM"))
        psum_p = ctx.enter_context(tc.tile_pool(name="psum_p", bufs=1, space="PSUM"))

        # ---- setup loads: scalar (ACT) HWDGE ring, so they bypass the flood's
        # sync-ring FIFO and complete while the first y-loads stream.
        ident_sb = const.tile([16, 16], F32, tag="ident")
        nc.scalar.dma_start(out=ident_sb, in_=ident[:, :])
        ht_sb = const.tile([NB, H], F32, tag="ht")
        nc.scalar.dma_start(out=ht_sb, in_=ht32[:, :])
        wst_sb = const.tile([P, 2, H], F32, tag="wst")  # W_score^T as [k, kk, h]
        nc.scalar.dma_start(out=wst_sb, in_=wst.rearrange("(kk p) h -> p kk h", p=P))
        watt_sb = const.tile([P, 4, OUT_D], F32, tag="watt")  # W_att as [d, dd, j]
        nc.scalar.dma_start(out=watt_sb, in_=watt.rearrange("(dd p) j -> p dd j", p=P))

        ones_row16 = const.tile([1, P], F16, tag="ones_row16")
        nc.vector.memset(ones_row16, 1.0)
        ones_col = const.tile([P, 1], F32, tag="ones_col")
        nc.vector.memset(ones_col, 1.0)
        ones1 = const.tile([1, 1], F32, tag="ones1")
        nc.vector.memset(ones1, 1.0)
        ones_row32 = const.tile([1, P], F32, tag="ones_row32")
        nc.vector.memset(ones_row32, 1.0)
        shift_col = const.tile([P, 1], F32, tag="shift_col")
        nc.vector.memset(shift_col, EXP_SHIFT)

        # ---- y-load flood: sync HWDGE ring, 32 half-batch f16 tiles --------
        # t = p*TT + i block mapping gives contiguous 4KB runs per partition
        # (softmax/context are t-permutation-invariant, so relabeling is free).
        ylist = {}

        def emit_load(k):
            b, v = divmod(k, 2)
            y = ybufs.tile([P, HT, H], F16, tag="y16", name=f"y16_{k}")
            src = hidden[b].rearrange("(p i) h -> p i h", i=TT)[
                :, v * HT : (v + 1) * HT, :
            ]
            nc.sync.dma_start(out=y, in_=src)
            ylist[k] = y

        PREF = 6  # half-tiles of DMA-ahead
        for k in range(PREF):
            emit_load(k)

        # ---- h_t^T and u = h_t @ W_score^T (full fp32) ---------------------
        htT_sb = const.tile([P, 2, NB], F32, tag="htT")  # h_t^T halves [k, half, b]
        for half in range(2):
            ps_tr = psum_t.tile([P, NB], F32, tag="ptmp", name=f"ps_tr{half}")
            nc.tensor.matmul(
                ps_tr,
                lhsT=ht_sb[:, half * P : (half + 1) * P],
                rhs=ident_sb,
                start=True,
                stop=True,
            )
            nc.scalar.copy(out=htT_sb[:, half, :], in_=ps_tr)

        ps_u = psum_t.tile([NB, H], F32, tag="ptmp")
        for half in range(2):
            nc.tensor.matmul(
                ps_u,
                lhsT=htT_sb[:, half, :],
                rhs=wst_sb[:, half, :],
                start=(half == 0),
                stop=(half == 1),
            )
        u16_sb = const.tile([NB, H], F16, tag="u16")
        nc.scalar.copy(out=u16_sb, in_=ps_u)
        # flatten u onto partition 0 so the per-batch broadcast matmul's rhs
        # has a legal base partition (small SBUF->SBUF DMA, scalar ring)
        u16_flat = const.tile([1, NB, H], F16, tag="u16_flat")
        nc.scalar.dma_start(out=u16_flat, in_=u16_sb)

        # ---- persistent PSUM accumulators ----------------------------------
        ctxT_ps = [
            psum_p.tile([P, NB], F32, tag=f"ctxT{j}", name=f"ctxT{j}")
            for j in range(2)
        ]
        # final output accumulator: do the h_t @ W_att half at setup time
        out_ps = psum_p.tile([NB, OUT_D], F32, tag="out_ps", name="out_ps")
        for dd in range(2, 4):
            nc.tensor.matmul(
                out_ps,
                lhsT=htT_sb[:, dd - 2, :],
                rhs=watt_sb[:, dd, :],
                start=(dd == 2),
                stop=False,
            )

        # ---- per-batch pipeline, tail stages skewed by one batch -----------
        q_all = const.tile([P, NB], F32, tag="q_all")
        state = {}

        def emit_batch_head(b):
            p_t = sc.tile([P, TT], BF16, tag="p", name=f"p{b}")
            ctx_ps = psum_t.tile([1, H], F32, tag="ptmp", name=f"ctx{b}")

            # broadcast u[b] to all partitions (PE ones-matmul), then
            # materialize it replicated HT times as a PLAIN f16 tile so the
            # score mul's in1 has no 0-stride dim (keeps DVE 2x mode).
            ubc_ps = psum_u.tile([P, H], F32, tag="ubc", name=f"ubc{b}")
            nc.tensor.matmul(
                ubc_ps,
                lhsT=ones_row16,
                rhs=u16_flat[:, b, :],
                start=True,
                stop=True,
            )
            ubc8 = sc.tile([P, HT, H], F16, tag="ubc8", name=f"ubc8_{b}")
            nc.scalar.copy(
                out=ubc8, in_=ubc_ps.unsqueeze(1).broadcast_to([P, HT, H])
            )

            for v in range(2):
                k = b * 2 + v
                if k + PREF < 2 * NB:
                    emit_load(k + PREF)
                y16 = ylist.pop(k)

                # score chain, all on DVE: mul (f16 2x packed), same-tensor
                # pair-adds (2-port + packed), reduce.  gpsimd stays COLD -
                # its SBUF port traffic would knock DVE out of packed mode.
                z = sc.tile([P, HT, H], F16, tag="z")
                nc.vector.tensor_mul(z, y16, ubc8)
                z1 = sc.tile([P, HT, 128], F16, tag="z1")
                nc.vector.tensor_add(z1, z[:, :, 0:128], z[:, :, 128:256])
                z2 = sc.tile([P, HT, 64], F16, tag="z2")
                nc.vector.tensor_add(z2, z1[:, :, 0:64], z1[:, :, 64:128])
                score = sc.tile([P, HT], F32, tag="score")
                nc.vector.tensor_reduce(
                    out=score, in_=z2, axis=mybir.AxisListType.X, op=add
                )

                # p = exp(score - 50) in bf16 (can't overflow)
                nc.scalar.activation(
                    out=p_t[:, v * HT : (v + 1) * HT],
                    in_=score,
                    func=mybir.ActivationFunctionType.Exp,
                    bias=shift_col,
                    scale=1.0,
                )

                # ctx += sum_i p[:, i] * y[:, i, :]  (unnormalized)
                for i in range(HT):
                    ii = v * HT + i
                    nc.tensor.matmul(
                        ctx_ps,
                        lhsT=p_t[:, ii : ii + 1],
                        rhs=y16[:, i, :],
                        start=(ii == 0),
                        stop=(ii == TT - 1),
                    )
            state[b] = (p_t, ctx_ps)

        def emit_batch_tail(b):
            # s = sum over partitions of q via a tiny PE ones-matmul (gpsimd
            # partition_all_reduce thrashes the pool config - avoid).
            # Normalization is folded into the scatter matmul's rhs.  Emitted
            # one batch late so these ops never sit ahead of the next batch's
            # streaming stages in any engine's instruction stream.
            p_t, ctx_ps = state.pop(b)
            nc.vector.tensor_reduce(
                out=q_all[:, b : b + 1], in_=p_t, axis=mybir.AxisListType.X,
                op=add,
            )
            ctx_row = sc.tile([1, H], F32, tag="ctx_row_sb")
            nc.scalar.copy(out=ctx_row, in_=ctx_ps)
            for j in range(2):
                nc.tensor.matmul(
                    ctxT_ps[j][:, b : b + 1],
                    lhsT=ctx_row[:, j * P : (j + 1) * P],
                    rhs=ones1,
                    start=True,
                    stop=True,
                )

        for b in range(NB):
            emit_batch_head(b)
            if b >= 1:
                emit_batch_tail(b - 1)
        emit_batch_tail(NB - 1)

        # ---- finalize: s per batch, normalize ctx^T, @ W_att, tanh ---------
        s_row_ps = psum_t.tile([1, NB], F32, tag="ptmp", name="s_row")
        nc.tensor.matmul(s_row_ps, lhsT=ones_col, rhs=q_all, start=True, stop=True)
        s_row = sc.tile([1, NB], F32, tag="s_row")
        nc.scalar.copy(out=s_row, in_=s_row_ps)
        rs_row = sc.tile([1, NB], F32, tag="rs_row")
        nc.vector.reciprocal(out=rs_row, in_=s_row)
        rs_ps = psum_u.tile([P, NB], F32, tag="ubc", name="rs_bc")
        nc.tensor.matmul(rs_ps, lhsT=ones_row32, rhs=rs_row, start=True, stop=True)
        rs_all = sc.tile([P, NB], F32, tag="rs_all")
        nc.scalar.copy(out=rs_all, in_=rs_ps)
        preT = sc.tile([P, 2, NB], F32, tag="preT")
        for j in range(2):
            nc.vector.tensor_mul(preT[:, j, :], ctxT_ps[j], rs_all)
        for dd in range(2):
            nc.tensor.matmul(
                out_ps,
                lhsT=preT[:, dd, :],
                rhs=watt_sb[:, dd, :],
                start=False,
                stop=(dd == 1),
            )
        out_sb = sc.tile([NB, OUT_D], F32, tag="out_sb")
        nc.scalar.activation(
            out=out_sb, in_=out_ps, func=mybir.ActivationFunctionType.Tanh
        )
        nc.sync.dma_start(out=out[:, :], in_=out_sb)


_NC_CACHE = {}


def _get_nc():
    if "nc" not in _NC_CACHE:
        nc = bacc.Bacc("TRN2", target_bir_lowering=False, debug=False)
        hidden = nc.declare_dram_parameter("hidden", [NB, T, H], F16, isOutput=False)
        ht32 = nc.declare_dram_parameter("ht32", [NB, H], F32, isOutput=False)
        wst = nc.declare_dram_parameter("w_score_t", [H, H], F32, isOutput=False)
        watt = nc.declare_dram_parameter("w_att", [2 * H, OUT_D], F32, isOutput=False)
        ident = nc.declare_dram_parameter("ident16", [16, 16], F32, isOutput=False)
        out = nc.declare_dram_parameter("out", [NB, OUT_D], F32, isOutput=True)
        with tile.TileContext(nc) as tc:
            _build_kernel(nc, tc, hidden, ht32, wst, watt, ident, out)
        nc.compile()
        _NC_CACHE["nc"] = nc
    return _NC_CACHE["nc"]


def _run(hidden_states, W_score, W_att, trace=False, trace_kwargs=None):
    hidden_states = np.asarray(hidden_states, dtype=np.float32)
    W_score = np.asarray(W_score, dtype=np.float32)
    W_att = np.ascontiguousarray(np.asarray(W_att, dtype=np.float32))
    hidden16 = np.ascontiguousarray(hidden_states.astype(np.float16))
    ht32 = np.ascontiguousarray(hidden_states[:, T - 1, :])
    wst = np.ascontiguousarray(W_score.T)
    ident = np.eye(16, dtype=np.float32)

    nc = _get_nc()
    in_maps = []
    for c in range(N_CORES):
        in_maps.append(
            {
                "hidden": hidden16[c * NB : (c + 1) * NB],
                "ht32": ht32[c * NB : (c + 1) * NB],
                "w_score_t": wst,
                "w_att": W_att,
                "ident16": ident,
            }
        )
    kwargs = {}
    if trace:
        kwargs["trace"] = True
        if trace_kwargs:
            kwargs.update(trace_kwargs)
    res = run_bass_kernel_spmd(nc, in_maps, list(range(N_CORES)), **kwargs)
    out = np.concatenate([res.results[c]["out"] for c in range(N_CORES)], axis=0)
    return out, res


def kernel(hidden_states, W_score, W_att):
    out, _ = _run(hidden_states, W_score, W_att, trace=False)
    return out
